# revision 1
# baseline (speedup 1.0000x reference)
"""Trainium2 Bass kernel for nn_DataTermLayer (data-term update of optical-flow).

Key observation: the reference's bilinear warp feeds *normalized* coords in
[-1, 1] straight into a pixel-space sampler, so after clipping the gather
only ever touches I1[b, 0:3, 0:3]. The whole layer therefore reduces to
elementwise math plus 9 per-image scalars:

  t2x = u + 2*w ; t2y = v + 2*h          (pre-division coords, f32-exact)
  nx  = t2x/511 - 1 ; ny = t2y/511 - 1   (device uses mult by r511)
  warped = [nx>=0 and ny>=0] * bilinear(P3x3, nx, ny)
  out_u = u - 0.1*(warped - I2)*(I1[h+1,w]-I1[h,w])
  out_v = v - 0.1*(warped - I2)*(I1[h,w+1]-I1[h,w])

Region split: warped is EXACTLY zero wherever 2w + u < 511 or 2h + v < 511
(the mask compare is done in pre-division space, bit-exact with the
reference's floor/clip branch decisions). With max|flow| ~ 5.4 that is all
cols < WZ~253 and all rows < HZ~253 -- ~74% of pixels only need
out = flow + 0.1*I2*grad. The full warp runs on the bottom-right quadrant
(col-sliced ops) and on a small 3-row "band" strip (rows HZ..255 of all
images gathered into one 16-partition tile).

The bilinear blend uses hat-basis coords mxp=min(t2*r511, 2),
e=relu(t2*r511-2) and per-image linear coefficients folded with the -0.1
scale on the host.

Sharding: pure data-parallel, 4 images per core across 8 cores.
"""
import sys

sys.path.insert(0, "/opt/trn_rl_repo")

import numpy as np

import concourse.bass as bass
import concourse.mybir as mybir
from concourse.bass_utils import run_bass_kernel_spmd
from concourse.tile import TileContext

F32 = mybir.dt.float32
ALU = mybir.AluOpType
ACTF = mybir.ActivationFunctionType

R511 = np.float32(1.0) / np.float32(511.0)
C1 = 511.0  # min f32 t with fl(t/511) >= 1  (verified exhaustively)


def build_nc(n_imgs: int = 4, n_rb: int = 4, wz: int = 253, hz: int = 253,
             legalize: bool = True):
    """One NeuronCore program: n_imgs images of [n_rb*128, 512].

    wz: first column where the warp can be nonzero (cols < wz are
    certainly zero-warp).  hz: same for rows.  Full-warp math runs on
    cols [wz, 512) of the bottom half plus a band strip rows [hz, 256).
    """
    assert n_rb == 4 and 225 <= hz <= 256 and 0 < wz <= 256
    W = 512
    H = n_rb * 128
    NBC = 256 - hz  # band compute rows per image (rows hz..255)
    NBR = NBC + 1   # band rows loaded per image (+1 for the row-shift grad)
    WF = W - wz     # full-math columns
    nc = bass.Bass()

    I1 = nc.dram_tensor("I1", [n_imgs, H + 1, W], F32, kind="ExternalInput")
    I2 = nc.dram_tensor("I2", [n_imgs, H, W], F32, kind="ExternalInput")
    FL = nc.dram_tensor("FL", [n_imgs, H, W, 2], F32, kind="ExternalInput")
    NCC = 9 * n_imgs + n_rb + 10  # +1 band gy2, +9 band-layout consts
    CC = nc.dram_tensor("CC", [128, NCC], F32, kind="ExternalInput")
    GX = nc.dram_tensor("GX", [128, 1024], F32, kind="ExternalInput")
    OUT = nc.dram_tensor("OUT", [n_imgs, H, W, 2], F32, kind="ExternalOutput")

    FDH = 1024  # free-dim of one half tile (2 row-blocks x 512)
    NBP = max(1, NBR * n_imgs)  # band partitions

    with TileContext(nc) as tc:
        with (
            tc.tile_pool(name="stat", bufs=1) as pstat,
            tc.tile_pool(name="pin", bufs=3) as pin,
            tc.tile_pool(name="ptmp", bufs=2) as ptmp,
            tc.tile_pool(name="pband", bufs=1) as pband,
        ):
            gx2 = pstat.tile([128, 1024], F32)
            nc.sync.dma_start(gx2[:], GX[:])
            cc = pstat.tile([128, NCC], F32)
            nc.sync.dma_start(cc[:], CC[:])
            cm2 = pstat.tile([128, 1], F32)
            nc.gpsimd.memset(cm2[:], -2.0)

            def cC(j):  # [128,1] column of cc
                return cc[:, j : j + 1]

            # ---------------- warp math on a generic tile set ---------------
            def warp_chain(pool, tag, P, fdims, t2x, t2y, i2v, bimg, dt_out):
                """Emit the warp pipeline writing 0.1*I2 + (-0.1)*zm*wr into
                dt_out.  t2x/t2y/i2v: APs with P partitions, fd free elems.
                bimg: image index for consts (None => band layout)."""
                cof = 9 * n_imgs + n_rb + 1

                def col(k, b):
                    c = cC(cof + k) if b is None else cC(9 * b + k)
                    return c[:P]

                shp = [P] + list(fdims)
                mxp = pool.tile(shp, F32, tag=f"{tag}mxp", name=f"{tag}mxp",
                                bufs=1)
                nc.vector.tensor_scalar(
                    mxp[:], t2x, float(R511), 2.0, ALU.mult, ALU.min
                )
                ex = pool.tile(shp, F32, tag=f"{tag}ex", name=f"{tag}ex", bufs=1)
                nc.scalar.activation(
                    ex[:], t2x, ACTF.Relu, bias=cm2[:P], scale=float(R511)
                )
                myp = pool.tile(shp, F32, tag=f"{tag}myp", name=f"{tag}myp", bufs=1)
                nc.vector.tensor_scalar(
                    myp[:], t2y, float(R511), 2.0, ALU.mult, ALU.min
                )
                ey = pool.tile(shp, F32, tag=f"{tag}ey", name=f"{tag}ey", bufs=1)
                nc.scalar.activation(
                    ey[:], t2y, ACTF.Relu, bias=cm2[:P], scale=float(R511)
                )
                lt = []
                for K in range(3):
                    # lt_K = mxp*beta'+alphat' and eg_K = ex*gamma' on ACT
                    # (interleaved so the DVE adds start early), add on DVE
                    ltK = pool.tile(shp, F32, tag=f"{tag}lt{K}",
                                    name=f"{tag}lt{K}", bufs=1)
                    nc.scalar.activation(
                        ltK[:], mxp[:], ACTF.Identity,
                        bias=col(3 * K + 1, bimg), scale=col(3 * K + 0, bimg),
                    )
                    eg = pool.tile(shp, F32, tag=f"{tag}eg",
                                   name=f"{tag}eg{K}", bufs=1)
                    nc.scalar.activation(
                        eg[:], ex[:], ACTF.Identity, bias=0.0,
                        scale=col(3 * K + 2, bimg),
                    )
                    nc.vector.tensor_tensor(ltK[:], eg[:], ltK[:], ALU.add)
                    lt.append(ltK)
                # wr = lin0 + (myp-1)*lin1 + ey*lin2 (into lt0), all on DVE to
                # avoid DVE->Pool->DVE ping-pong in the chain tail
                nc.vector.scalar_tensor_tensor(
                    lt[1][:], myp[:], 1.0, lt[1][:], ALU.subtract, ALU.mult
                )
                nc.vector.tensor_tensor(lt[2][:], ey[:], lt[2][:], ALU.mult)
                nc.vector.tensor_tensor(lt[0][:], lt[0][:], lt[1][:], ALU.add)
                nc.vector.tensor_tensor(lt[0][:], lt[0][:], lt[2][:], ALU.add)
                # masks + data term
                nc.vector.scalar_tensor_tensor(
                    lt[0][:], t2x, C1, lt[0][:], ALU.is_ge, ALU.mult
                )
                nc.vector.scalar_tensor_tensor(
                    lt[0][:], t2y, C1, lt[0][:], ALU.is_ge, ALU.mult
                )
                nc.vector.scalar_tensor_tensor(
                    dt_out, i2v, 0.1, lt[0][:], ALU.mult, ALU.add
                )

            # ---------------- band strip (rows hz..255, all imgs) -----------
            if NBC > 0:
                bi1 = pband.tile([NBP, 512], F32)
                bi1r = pband.tile([NBP, 512], F32)
                bi2 = pband.tile([NBP, 512], F32)
                bfl = pband.tile([NBP, 512, 2], F32)
                for b in range(n_imgs):
                    bsl = slice(NBR * b, NBR * (b + 1))
                    nc.sync.dma_start(bi1[bsl, :], I1[b, hz : hz + NBR, :])
                    nc.sync.dma_start(
                        bi1r[bsl, :], I1[b, hz + 1 : hz + 1 + NBR, :]
                    )
                    nc.sync.dma_start(bi2[bsl, :], I2[b, hz : hz + NBR, :])
                    nc.sync.dma_start(
                        bfl[bsl, :, :], FL[b, hz : hz + NBR, :, :]
                    )
                bu = bfl[:, :, 0]
                bv = bfl[:, :, 1]
                bt2x = pband.tile([NBP, 512], F32)
                nc.vector.tensor_tensor(bt2x[:], bu, gx2[:NBP, 0:512], ALU.add)
                bt2y = pband.tile([NBP, 512], F32)
                nc.scalar.activation(
                    bt2y[:], bv, ACTF.Identity,
                    bias=cC(9 * n_imgs + n_rb)[:NBP], scale=1.0,
                )
                bdt = pband.tile([NBP, 512], F32)
                warp_chain(pband, "bnd", NBP, [512], bt2x[:], bt2y[:],
                           bi2[:], None, bdt[:])
                bg1 = pband.tile([NBP, 512], F32)
                nc.vector.tensor_tensor(bg1[:], bi1r[:], bi1[:], ALU.subtract)
                bg2 = pband.tile([NBP, 512], F32)
                nc.vector.tensor_tensor(
                    bg2[:, 0:511], bi1[:, 1:512], bi1[:, 0:511], ALU.subtract
                )
                nc.gpsimd.memset(bg2[:, 511:512], 0.0)
                bmu = pband.tile([NBP, 512], F32)
                nc.gpsimd.tensor_tensor(bmu[:], bdt[:], bg1[:], ALU.mult)
                nc.vector.tensor_tensor(bu, bu, bmu[:], ALU.add)
                nc.gpsimd.tensor_tensor(bg2[:], bdt[:], bg2[:], ALU.mult)
                nc.vector.tensor_tensor(bv, bv, bg2[:], ALU.add)

            # ---------------- per image ------------------------------------
            for b in range(n_imgs):
                i1 = pin.tile([128, n_rb * 512], F32, tag="i1")
                nc.sync.dma_start(
                    i1[:].rearrange("p (rb w) -> p rb w", rb=n_rb),
                    I1[b, 0:H, :].rearrange("(rb p) w -> p rb w", p=128),
                )
                i1r = pin.tile([128, n_rb * 512], F32, tag="i1r")
                nc.sync.dma_start(
                    i1r[:].rearrange("p (rb w) -> p rb w", rb=n_rb),
                    I1[b, 1 : H + 1, :].rearrange("(rb p) w -> p rb w", p=128),
                )
                i2 = pin.tile([128, n_rb * 512], F32, tag="i2")
                nc.sync.dma_start(
                    i2[:].rearrange("p (rb w) -> p rb w", rb=n_rb),
                    I2[b].rearrange("(rb p) w -> p rb w", p=128),
                )
                fl = pin.tile([128, n_rb * 512, 2], F32, tag="fl")
                nc.sync.dma_start(
                    fl[:].rearrange("p (rb w) c -> p rb w c", rb=n_rb),
                    FL[b].rearrange("(rb p) w c -> p rb w c", p=128),
                )

                for hi in range(2):
                    hs = hi * FDH
                    hsl = slice(hs, hs + FDH)
                    u = fl[:, hsl, 0]
                    v = fl[:, hsl, 1]
                    i1h = i1[:, hsl]
                    i1rh = i1r[:, hsl]
                    i2h = i2[:, hsl]

                    g2 = ptmp.tile([128, FDH], F32, tag="g2", bufs=2)
                    nc.vector.tensor_tensor(
                        g2[:, 0:1023],
                        i1[:, hs + 1 : hs + 1024],
                        i1[:, hs : hs + 1023],
                        ALU.subtract,
                    )
                    g1 = ptmp.tile([128, FDH], F32, tag="g1", bufs=2)
                    nc.vector.tensor_tensor(g1[:], i1rh, i1h, ALU.subtract)
                    nc.gpsimd.memset(g2[:, 511:1024:512], 0.0)

                    dt = ptmp.tile([128, FDH], F32, tag="dt", bufs=2)
                    if hi == 0:
                        # top half: warp certainly zero -> dt = 0.1*I2
                        nc.vector.tensor_scalar_mul(dt[:], i2h, 0.1)
                    else:
                        # zero-warp columns
                        i2r = i2h.rearrange("p (r w) -> p r w", r=2)
                        dtr = dt[:].rearrange("p (r w) -> p r w", r=2)
                        nc.vector.tensor_scalar_mul(
                            dtr[:, :, 0:wz], i2r[:, :, 0:wz], 0.1
                        )
                        # full-math columns
                        ur = u.rearrange("p (r w) -> p r w", r=2)[:, :, wz:]
                        vr = v.rearrange("p (r w) -> p r w", r=2)[:, :, wz:]
                        i2f = i2r[:, :, wz:]
                        gxf = gx2[:].rearrange("p (r w) -> p r w", r=2)[
                            :, :, wz:
                        ]
                        t2x = ptmp.tile([128, 2, WF], F32, tag="t2x", bufs=1)
                        nc.vector.tensor_tensor(t2x[:], ur, gxf, ALU.add)
                        t2y = ptmp.tile([128, 2, WF], F32, tag="t2y", bufs=1)
                        for rbl in range(2):
                            nc.scalar.activation(
                                t2y[:, rbl, :], vr[:, rbl, :], ACTF.Identity,
                                bias=cC(9 * n_imgs + 2 + rbl), scale=1.0,
                            )
                        dtf = dtr[:, :, wz:]
                        warp_chain(ptmp, "f", 128, [2, WF], t2x[:], t2y[:],
                                   i2f, b, dtf)

                    # flow update (in place into fl tile)
                    nc.gpsimd.tensor_tensor(g1[:], dt[:], g1[:], ALU.mult)
                    nc.vector.tensor_tensor(u, u, g1[:], ALU.add)
                    nc.gpsimd.tensor_tensor(g2[:], dt[:], g2[:], ALU.mult)
                    nc.vector.tensor_tensor(v, v, g2[:], ALU.add)

                # patch band rows (overwrites the zero-branch values there)
                if NBC > 0:
                    nc.sync.dma_start(
                        fl[hz - 128 : hz - 128 + NBC, 512:1024, :],
                        bfl[NBR * b : NBR * b + NBC, :, :],
                    )

                for hi in range(2):
                    nc.sync.dma_start(
                        OUT[b, hi * 256 : hi * 256 + 256].rearrange(
                            "(rb p) w c -> p rb w c", p=128
                        ),
                        fl[:, hi * 1024 : hi * 1024 + 1024, :].rearrange(
                            "p (rb w) c -> p rb w c", rb=2
                        ),
                    )
    if legalize:
        legalize_single_wait(nc)
    return nc


# ---------------------------------------------------------------------------
# Post-pass: this walrus build encodes a single sync-wait slot per TPB
# instruction. Tile's sem assignment can emit 2+ waits on one instruction;
# hoist all but the last wait onto same-engine EventSemaphore carriers placed
# immediately before it (the sequencer then waits sequentially, which is
# semantically identical).
def legalize_single_wait(nc):
    import bass_rust

    capped = {
        mybir.EngineType.Activation,
        mybir.EngineType.DVE,
        mybir.EngineType.Pool,
        mybir.EngineType.PE,
        mybir.EngineType.SP,
    }
    exempt = {"EventSemaphore", "NoOp", "TriggerDma"}
    n = 0
    for fn in nc.m.functions:
        for blk in fn.blocks:
            insts = blk.instructions  # live list
            rebuilt = []
            changed = False
            for inst in list(insts):
                si = inst.sync_info
                waits = list(si.on_wait) if si is not None else []
                if (
                    len(waits) > 1
                    and inst.engine in capped
                    and str(inst.opcode) not in exempt
                ):
                    for w in waits[:-1]:
                        ev = mybir.InstEventSemaphore(
                            name=f"waitcarrier_{inst.name}_{n}", ins=[], outs=[]
                        )
                        ev.engine = inst.engine
                        ev.sync_info = bass_rust.SyncInfo(
                            on_wait=[w], on_update=[]
                        )
                        rebuilt.append(ev)
                        n += 1
                    inst.sync_info = bass_rust.SyncInfo(
                        on_wait=[waits[-1]], on_update=list(si.on_update)
                    )
                    changed = True
                rebuilt.append(inst)
            if changed:
                insts[:] = rebuilt
    return n


def host_consts(I1c: np.ndarray, n_rb: int = 4, hz: int = 253) -> np.ndarray:
    """Per-image folded warp coefficients + per-partition 2*h columns.

    I1c: [n_imgs, H, W] float32.  Returns [128, 9*n_imgs + n_rb + 10] f32.
    Per image b, cols 9*b+3*K+(0:beta', 1:alphat', 2:gamma').
    Col 9n+rb: 2*(128*rb+p).  Col 9n+n_rb: band 2*h.  Cols 9n+n_rb+1..+9:
    band-partition-layout consts (partition 4b+r holds image b's values).
    """
    f = np.float32
    n_imgs = I1c.shape[0]
    cc = np.zeros((128, 9 * n_imgs + n_rb + 10), dtype=np.float32)
    m01 = f(-0.1)
    allc = np.zeros((n_imgs, 9), dtype=np.float32)
    for b in range(n_imgs):
        P = I1c[b, 0:3, 0:3].astype(np.float32)
        d1 = (P[:, 1] - P[:, 0]).astype(f)
        d2 = (P[:, 2] - P[:, 1]).astype(f)
        alpha = np.array(
            [P[0, 0], f(P[1, 0] - P[0, 0]), f(P[2, 0] - P[1, 0])], dtype=f
        )
        beta = np.array([d1[0], f(d1[1] - d1[0]), f(d1[2] - d1[1])], dtype=f)
        gamma = np.array([d2[0], f(d2[1] - d2[0]), f(d2[2] - d2[1])], dtype=f)
        for K in range(3):
            allc[b, 3 * K + 0] = f(m01 * beta[K])
            allc[b, 3 * K + 1] = f(m01 * f(alpha[K] - beta[K]))
            allc[b, 3 * K + 2] = f(m01 * gamma[K])
        cc[:, 9 * b : 9 * b + 9] = allc[b][None, :]
    p = np.arange(128, dtype=np.float32)
    for rb in range(n_rb):
        cc[:, 9 * n_imgs + rb] = f(2.0) * (f(128.0 * rb) + p)
    # band columns (NBR = 257-hz rows per image)
    base = 9 * n_imgs + n_rb
    nbr = 257 - hz
    for b in range(n_imgs):
        for r in range(nbr):
            pp = nbr * b + r
            if pp < 128:
                cc[pp, base] = f(2.0) * f(hz + r)
                cc[pp, base + 1 : base + 10] = allc[b]
    return cc


def host_gx() -> np.ndarray:
    w2 = (np.float32(2.0) * np.arange(512, dtype=np.float32)).astype(np.float32)
    return np.tile(w2, (128, 2)).astype(np.float32)


_NC = None
_NC_KEY = None


def _get_nc(wz, hz):
    global _NC, _NC_KEY
    if _NC is None or _NC_KEY != (wz, hz):
        _NC = build_nc(4, 4, wz=wz, hz=hz)
        _NC_KEY = (wz, hz)
    return _NC


def _splits(flow):
    umax = float(max(flow[..., 0].max(), 0.0))
    vmax = float(max(flow[..., 1].max(), 0.0))
    # first col/row where 2*x + d can reach 511.0 (f32-exact threshold)
    wz = int(min(256, max(1, (511.0 - umax) // 2 + 1)))
    hz = int(min(256, max(225, (511.0 - vmax) // 2 + 1)))
    # paranoia: verify in f32 exactly like the device compare
    assert np.float32(2.0 * (wz - 1)) + np.float32(umax) < np.float32(511.0)
    assert np.float32(2.0 * (hz - 1)) + np.float32(vmax) < np.float32(511.0)
    return wz, hz


def _make_in_maps(I1, I2, flow, wz, hz, n_cores=8):
    per = I1.shape[0] // n_cores
    gx = host_gx()
    in_maps = []
    for c in range(n_cores):
        sl = slice(c * per, (c + 1) * per)
        i1c = np.ascontiguousarray(I1[sl, :, :, 0], dtype=np.float32)
        i1pad = np.concatenate([i1c, i1c[:, -1:, :]], axis=1)
        in_maps.append(
            {
                "I1": np.ascontiguousarray(i1pad),
                "I2": np.ascontiguousarray(I2[sl, :, :, 0], dtype=np.float32),
                "FL": np.ascontiguousarray(flow[sl], dtype=np.float32),
                "CC": host_consts(i1c, 4, hz),
                "GX": gx,
            }
        )
    return in_maps


def run(I1, I2, flow, trace=False, **kw):
    wz, hz = _splits(np.asarray(flow))
    nc = _get_nc(wz, hz)
    in_maps = _make_in_maps(I1, I2, flow, wz, hz)
    res = run_bass_kernel_spmd(nc, in_maps, list(range(8)), trace=trace, **kw)
    out = np.concatenate([r["OUT"] for r in res.results], axis=0)
    return out, res


def kernel(I1, I2, flow):
    out, _ = run(I1, I2, flow)
    return out.astype(np.float32)

